# revision 26
# baseline (speedup 1.0000x reference)
"""Trainium2 Bass kernel for nn_MultiHeadAttention_72816875536681.

Module math (see harness reference):
  residual = q
  qn = LayerNorm(q) (pre-LN, q only)
  Q = (qn @ Wq.T)  -> view [b, l, DK=64, NH=8] -> heads axis is DK (64 heads of dim 8)
  K = (k @ Wk.T), V = (v @ Wv.T) likewise
  scores[b,h,q,k] = QK^T/sqrt(64)... (temp = DK**0.5 = 8) + similarity
  attn = softmax(scores, -1)            (returned as output 2)
  out = concat_heads(attn @ V) @ Wfc.T + bfc + residual   (returned as output 1)

Sharding (8 cores): batch (4-way) x head-group (2-way tensor parallel).
Core c handles batch bi = c//2 and dk-heads [32*g, 32*g+32), g = c%2.
The fc projection is row-sharded; partial outputs are summed on host
(the "all-reduce"), attn shards are concatenated on host.

Device-side layout tricks:
 - All matmuls in float32r (full-rate single-pass PE mode; fp32 storage).
 - Heads are processed in quad-groups of 4, padded to partition offsets
   {0,32,64,96} so QK^T (contraction dim 8) packs 4 concurrent matmuls
   into the 128x128 PE array via row-group tiling, and A@V packs 4
   concurrent matmuls via col-group tiling.
 - similarity is added to QK^T directly in PSUM via an identity-matmul
   accumulation (PE does the add; no vector pass).
 - softmax skips max-subtraction (scores are provably small: |s| < ~7),
   exp runs on the scalar engine with fused row-sum accumulation (Z),
   division by Z is a per-partition tensor_scalar on the vector engine.
 - attn tiles are transposed on the PE (128x128 blocks -> PSUM) to build
   attn^T for the A@V contraction; PSUM->SBUF moves are single [128,512]
   vector copies.
"""

import numpy as np

import concourse.bass as bass
import concourse.mybir as mybir
import concourse.tile as tile
from concourse.bass_utils import run_bass_kernel_spmd
from concourse.masks import make_identity

# Problem constants (hardcoded per harness contract)
B, L, DM = 4, 512, 512
NH, DK = 8, 64
EPS = 1e-6
TEMP = float(DK) ** 0.5  # 8.0

NCORES = 8
HG = 2            # head-group shards (tensor parallel)
HL = DK // HG     # 32 local dk-heads per core
NQUAD = HL // 4   # 8 quad-groups of 4 heads
NT = L // 128     # 4 token tiles
NC_ = DM // 128   # 4 dm chunks

F32 = mybir.dt.float32
F32R = mybir.dt.float32r
AX = mybir.AxisListType
ALU = mybir.AluOpType
ACT = mybir.ActivationFunctionType

# Set by test harness to collect profiling info; grading path leaves these.
TRACE = False
TRACE_CORES = None
LAST_RESULT = {}


def _emit(nc):
    """Emit the single-core SPMD program."""
    # ---- DRAM I/O ----
    qT = nc.dram_tensor("qT", [DM, L], F32, kind="ExternalInput")
    kT = nc.dram_tensor("kT", [DM, L], F32R, kind="ExternalInput")
    vT = nc.dram_tensor("vT", [DM, L], F32R, kind="ExternalInput")
    simx = nc.dram_tensor("simx", [HL, L, L], F32R, kind="ExternalInput")
    wq = nc.dram_tensor("wq", [DM, NQUAD, 128], F32R, kind="ExternalInput")
    wk = nc.dram_tensor("wk", [DM, NQUAD, 128], F32R, kind="ExternalInput")
    wv = nc.dram_tensor("wv", [DM, HL, 32], F32R, kind="ExternalInput")
    wfc = nc.dram_tensor("wfc", [NQUAD, 128, DM], F32R, kind="ExternalInput")
    lng = nc.dram_tensor("lng", [DM], F32, kind="ExternalInput")
    lnb = nc.dram_tensor("lnb", [DM], F32, kind="ExternalInput")
    bfc = nc.dram_tensor("bfc", [DM], F32, kind="ExternalInput")
    resid = nc.dram_tensor("resid", [DM, L], F32, kind="ExternalInput")
    attn_o = nc.dram_tensor("attn_o", [HL, L, L], F32, kind="ExternalOutput")
    fc_o = nc.dram_tensor("fc_o", [DM, L], F32, kind="ExternalOutput")

    with tile.TileContext(nc) as tc:
        with (
            tc.tile_pool(name="const", bufs=1) as const,
            tc.tile_pool(name="persist", bufs=1) as persist,
        ):
            # ---- constants ----
            i128 = const.tile([128, 128], F32)
            make_identity(nc, i128[:, :])
            i128r = const.tile([128, 128], F32R)
            nc.vector.tensor_copy(out=i128r[:, :], in_=i128[:, :])
            ones1 = const.tile([1, 128], F32)
            nc.vector.memset(ones1[:, :], 1.0)
            onesd = const.tile([128, 1], F32)
            nc.vector.memset(onesd[:, :], 1.0 / DM)
            eps1 = const.tile([1, 1], F32)
            nc.vector.memset(eps1[:, :], EPS)
            lng_sb = const.tile([128, NC_], F32)
            nc.sync.dma_start(out=lng_sb[:, :], in_=lng[:].rearrange("(c p) -> p c", p=128))
            lnb_sb = const.tile([128, NC_], F32)
            nc.sync.dma_start(out=lnb_sb[:, :], in_=lnb[:].rearrange("(c p) -> p c", p=128))
            bfc_sb = const.tile([128, NC_], F32)
            nc.sync.dma_start(out=bfc_sb[:, :], in_=bfc[:].rearrange("(c p) -> p c", p=128))

            # ---- fc weights (persist; projection weights live in the proj scope) ----
            wfc_sb = persist.tile([128, NQUAD, DM], F32R)
            nc.sync.dma_start(out=wfc_sb[:, :, :], in_=wfc[:, :, :].rearrange("g p r -> p g r"))

            # persistent transposed projections (padded head layout)
            QT_sb = persist.tile([128, NQUAD, L], F32R)
            KT_sb = persist.tile([128, NQUAD, L], F32R)
            V_sb = persist.tile([128, NT, HL * 32], F32R)
            fcin = persist.tile([128, NQUAD, L], F32R)

            with (
                tc.tile_pool(name="wpad", bufs=1) as wpadp,
                tc.tile_pool(name="xin", bufs=1) as xin,
                tc.tile_pool(name="stage", bufs=2) as stage,
                tc.tile_pool(name="rows", bufs=1) as rows,
                tc.tile_pool(name="ps_stat", bufs=1, space="PSUM") as ps_stat,
                tc.tile_pool(name="ps_bcast", bufs=1, space="PSUM") as ps_bcast,
                tc.tile_pool(name="ps_proj", bufs=2, space="PSUM") as ps_proj,
            ):
                wq_pad = wpadp.tile([128, NC_, NQUAD, 128], F32R)
                nc.sync.dma_start(out=wq_pad[:, :, :, :],
                                  in_=wq[:, :, :].rearrange("(c p) g j -> p c g j", p=128))
                wk_pad = wpadp.tile([128, NC_, NQUAD, 128], F32R)
                nc.sync.dma_start(out=wk_pad[:, :, :, :],
                                  in_=wk[:, :, :].rearrange("(c p) g j -> p c g j", p=128))
                wv_sb = wpadp.tile([128, NC_, HL * 32], F32R)
                nc.sync.dma_start(out=wv_sb[:, :, :],
                                  in_=wv[:, :, :].rearrange("(c p) h e -> p c (h e)", p=128))
                qT_sb = xin.tile([128, NC_, L], F32)
                nc.sync.dma_start(out=qT_sb[:, :, :], in_=qT[:, :].rearrange("(c p) t -> p c t", p=128))
                kT_sb = xin.tile([128, NC_, L], F32R)
                nc.sync.dma_start(out=kT_sb[:, :, :], in_=kT[:, :].rearrange("(c p) t -> p c t", p=128))
                vT_sb = xin.tile([128, NC_, L], F32R)
                nc.sync.dma_start(out=vT_sb[:, :, :], in_=vT[:, :].rearrange("(c p) t -> p c t", p=128))

                # ---- LayerNorm stats for q: mean and 1/std per token, as [1, L] rows ----
                ps_mu = ps_stat.tile([1, L], F32)
                ps_sq = ps_stat.tile([1, L], F32)
                for dc in range(NC_):
                    q2 = stage.tile([128, L], F32, tag="q2")
                    nc.scalar.square(q2[:, :], qT_sb[:, dc, :])
                    nc.tensor.matmul(ps_mu[:, :], onesd[:, :], qT_sb[:, dc, :],
                                     start=(dc == 0), stop=(dc == NC_ - 1))
                    nc.tensor.matmul(ps_sq[:, :], onesd[:, :], q2[:, :],
                                     start=(dc == 0), stop=(dc == NC_ - 1))
                mu_row = rows.tile([1, L], F32)
                nc.vector.tensor_copy(out=mu_row[:, :], in_=ps_mu[:, :])
                var_row = rows.tile([1, L], F32)
                nc.vector.tensor_mul(out=var_row[:, :], in0=mu_row[:, :], in1=mu_row[:, :])
                nc.vector.tensor_sub(out=var_row[:, :], in0=ps_sq[:, :], in1=var_row[:, :])
                # std = sqrt(var + eps); isd = 1/std
                nc.scalar.activation(var_row[:, :], var_row[:, :], ACT.Sqrt, bias=eps1[:, :], scale=1.0)
                isd_row = rows.tile([1, L], F32)
                nc.vector.reciprocal(isd_row[:, :], var_row[:, :])

                # broadcast to [128, L]
                ps_MU = ps_bcast.tile([128, L], F32)
                nc.tensor.matmul(ps_MU[:, :], ones1[:, :], mu_row[:, :])
                ps_ISD = ps_bcast.tile([128, L], F32)
                nc.tensor.matmul(ps_ISD[:, :], ones1[:, :], isd_row[:, :])

                # qnT = (qT - MU) * ISD * g[d] + b[d]
                qnT_sb = xin.tile([128, NC_, L], F32R)
                for dc in range(NC_):
                    t1 = stage.tile([128, L], F32, tag="t1")
                    nc.vector.scalar_tensor_tensor(
                        out=t1[:, :], in0=qT_sb[:, dc, :], scalar=1.0,
                        in1=ps_MU[:, :], op0=ALU.mult, op1=ALU.subtract)
                    nc.vector.tensor_mul(out=t1[:, :], in0=t1[:, :], in1=ps_ISD[:, :])
                    nc.scalar.activation(qnT_sb[:, dc, :], t1[:, :], ACT.Identity,
                                         bias=lnb_sb[:, dc:dc + 1], scale=lng_sb[:, dc:dc + 1])

                # ---- projections ----
                for gi in range(NQUAD):
                    pq = ps_proj.tile([128, L], F32, tag="pp")
                    for dc in range(NC_):
                        nc.tensor.matmul(pq[:, :], wq_pad[:, dc, gi, :],
                                         qnT_sb[:, dc, :],
                                         start=(dc == 0), stop=(dc == NC_ - 1))
                    nc.vector.tensor_copy(out=QT_sb[:, gi, :], in_=pq[:, :])
                    pk = ps_proj.tile([128, L], F32, tag="pp")
                    for dc in range(NC_):
                        nc.tensor.matmul(pk[:, :], wk_pad[:, dc, gi, :],
                                         kT_sb[:, dc, :],
                                         start=(dc == 0), stop=(dc == NC_ - 1))
                    nc.vector.tensor_copy(out=KT_sb[:, gi, :], in_=pk[:, :])
                for tt in range(NT):
                    for vh in range(2):
                        pv = ps_proj.tile([128, 512], F32, tag="pv")
                        for dc in range(NC_):
                            nc.tensor.matmul(pv[:, :],
                                             vT_sb[:, dc, tt * 128:(tt + 1) * 128],
                                             wv_sb[:, dc, vh * 512:(vh + 1) * 512],
                                             start=(dc == 0), stop=(dc == NC_ - 1))
                        nc.vector.tensor_copy(out=V_sb[:, tt, vh * 512:(vh + 1) * 512],
                                              in_=pv[:, :])

            # ---- main attention loop ----
            with (
                tc.tile_pool(name="sim", bufs=3) as simp,
                tc.tile_pool(name="attn", bufs=3) as attnp,
                tc.tile_pool(name="zz", bufs=3) as zzp,
                tc.tile_pool(name="pth", bufs=2) as pthp,
                tc.tile_pool(name="ps_s", bufs=4, space="PSUM") as ps_s,
                tc.tile_pool(name="ps_t", bufs=2, space="PSUM") as ps_t,
                tc.tile_pool(name="ps_av", bufs=1, space="PSUM") as ps_av,
            ):
                for gi in range(NQUAD):
                    for half in range(2):
                        pth = [pthp.tile([128, NT, 256], F32R, tag=f"pth{m}", name=f"pth{m}")
                               for m in range(4)]
                        for qt2 in range(2):
                            qt = half * 2 + qt2
                            qs = slice(qt * 128, (qt + 1) * 128)
                            sim4 = simp.tile([128, 4, L], F32R, tag="sim4")
                            nc.sync.dma_start(
                                out=sim4[:, :, :],
                                in_=simx[4 * gi:4 * gi + 4, qs, :].rearrange("h q k -> q h k"))
                            attn4 = attnp.tile([128, 4, L], F32, tag="attn4")
                            z4 = zzp.tile([128, 4], F32, tag="z4")
                            iz4 = zzp.tile([128, 4], F32, tag="iz4")
                            for m in range(4):
                                ps = ps_s.tile([128, L], F32, tag="s")
                                nc.tensor.matmul(
                                    ps[:, :],
                                    QT_sb[32 * m:32 * m + 8, gi, qs],
                                    KT_sb[32 * m:32 * m + 8, gi, :],
                                    start=True, stop=False, tile_position=(32 * m, 0))
                                nc.tensor.matmul(
                                    ps[:, :], i128r[:, :],
                                    sim4[:, m, :],
                                    start=False, stop=True)
                                nc.scalar.activation(attn4[:, m, :], ps[:, :], ACT.Exp,
                                                     accum_out=z4[:, m:m + 1])
                            nc.vector.reciprocal(iz4[:, :], z4[:, :])
                            for m in range(4):
                                nc.vector.tensor_scalar_mul(attn4[:, m, :], attn4[:, m, :],
                                                            iz4[:, m:m + 1])
                            nc.sync.dma_start(
                                out=attn_o[4 * gi:4 * gi + 4, qs, :].rearrange("h q k -> q h k"),
                                in_=attn4[:, :, :])
                            # transpose each head's [128 q, 512 k] tile into [k, q] chunks
                            for m in range(4):
                                pt4 = ps_t.tile([128, NT, 128], F32, tag="pt4")
                                for c in range(NT):
                                    nc.tensor.transpose(pt4[:, c, :],
                                                        attn4[:, m, c * 128:(c + 1) * 128],
                                                        i128[:, :])
                                nc.vector.tensor_copy(
                                    out=pth[m][:, :, qt2 * 128:(qt2 + 1) * 128],
                                    in_=pt4[:, :, :])
                        # A @ V for this half (256 q columns), 4 heads packed in col groups
                        # fp32r matmuls require dst partition base 0, so the 4
                        # heads land in separate free-dim slots of one psum tile
                        pav = ps_av.tile([32, 4, 256], F32, tag="pav")
                        for m in range(4):
                            for c in range(NT):
                                hl = 4 * gi + m
                                nc.tensor.matmul(
                                    pav[0:32, m, :],
                                    V_sb[:, c, 32 * hl:32 * hl + 32],
                                    pth[m][:, c, :],
                                    start=(c == 0), stop=(c == NT - 1))
                        for m in range(4):
                            nc.vector.tensor_copy(
                                out=fcin[32 * m:32 * m + 32, gi, half * 256:(half + 1) * 256],
                                in_=pav[0:32, m, :])

            # ---- fc output projection (row-sharded partial) + bias + residual ----
            with (
                tc.tile_pool(name="resid", bufs=1) as residp,
                tc.tile_pool(name="fcout", bufs=2) as fcoutp,
                tc.tile_pool(name="ps_fc", bufs=2, space="PSUM") as ps_fc,
            ):
                resid_sb = residp.tile([128, NC_, L], F32)
                nc.sync.dma_start(out=resid_sb[:, :, :],
                                  in_=resid[:, :].rearrange("(c p) t -> p c t", p=128))
                for rt in range(NC_):
                    pf = ps_fc.tile([128, L], F32, tag="pf")
                    for gq in range(NQUAD):
                        nc.tensor.matmul(pf[:, :],
                                         wfc_sb[:, gq, rt * 128:(rt + 1) * 128],
                                         fcin[:, gq, :],
                                         start=(gq == 0), stop=(gq == NQUAD - 1))
                    ft = fcoutp.tile([128, L], F32, tag="ft")
                    nc.vector.scalar_tensor_tensor(
                        out=ft[:, :], in0=pf[:, :], scalar=bfc_sb[:, rt:rt + 1],
                        in1=resid_sb[:, rt, :], op0=ALU.add, op1=ALU.add)
                    nc.sync.dma_start(out=fc_o[rt * 128:(rt + 1) * 128, :], in_=ft[:, :])
    return nc


_NC_CACHE = []

# This container's walrus enforces small per-instruction sync-wait budgets
# (Matmult carries its waits on the LDWEIGHTS struct, which fits only one).
# Tile emits up to ~4 waits per instruction, so after scheduling we move the
# excess onto standalone InstEventSemaphore prefixes (<=2 waits each), which
# walrus accepts. Executed on the same engine queue, so semantics (engine
# stalls until conditions hold before the instruction) are identical.
_WAIT_BUDGET = {}
_WAIT_BUDGET_DEFAULT = 1
_WAIT_EXEMPT = {"EventSemaphore", "UnconditionalBranch", "Call", "ISA"}


def _legalize_waits(nc):
    count = 0
    for f in nc.m.functions:
        for blk in f.blocks:
            new_insts = []
            for inst in blk.instructions:
                si = inst.sync_info
                waits = list(si.on_wait) if (si is not None and si.on_wait) else []
                opcode = str(inst.opcode)
                if opcode in _WAIT_EXEMPT or len(waits) <= _WAIT_BUDGET.get(
                        opcode, _WAIT_BUDGET_DEFAULT):
                    new_insts.append(inst)
                    continue
                budget = _WAIT_BUDGET.get(opcode, _WAIT_BUDGET_DEFAULT)
                keep = waits[-budget:] if budget else []
                excess = waits[:-budget] if budget else waits
                for ci in range(0, len(excess), 2):
                    es = mybir.InstEventSemaphore(name=f"eswait_{count}")
                    count += 1
                    es.engine = inst.engine
                    es.sync_info = mybir.SyncInfo(
                        on_wait=excess[ci:ci + 2], on_update=[])
                    new_insts.append(es)
                si.on_wait = keep
                new_insts.append(inst)
            blk.instructions = new_insts
    return count


def build_nc():
    if not _NC_CACHE:
        nc = bass.Bass(trn_type="TRN2")
        _emit(nc)
        _legalize_waits(nc)
        _NC_CACHE.append(nc)
    return _NC_CACHE[0]


def _pad_qk(Wt):
    """Wt: [DM, 256] transposed projection weight (col j = 32*gi + 8*m + e).
    Returns [DM, NQUAD, 128] with cols padded to 32*m + e inside each gi block."""
    pad = np.zeros((DM, NQUAD, 4, 32), np.float32)
    pad[:, :, :, :8] = Wt.reshape(DM, NQUAD, 4, 8)
    return np.ascontiguousarray(pad.reshape(DM, NQUAD, 128))


def _pad_v(Wt):
    """Wt: [DM, 256] transposed V weight (col = 8*hl + e). Returns
    [DM, HL, 32] with head hl's 8 cols at [hl, :8], zeros elsewhere."""
    pad = np.zeros((DM, HL, 32), np.float32)
    pad[:, :, :8] = Wt.reshape(DM, HL, 8)
    return np.ascontiguousarray(pad)


def _pad_wfc(W):
    """W: [DM, 256] fc weight cols for this core (c_local = 32*gi + 8*m + e).
    Returns [NQUAD, 128, DM] with head (gi, m) rows at 32*m+e inside chunk gi,
    zero padding elsewhere (matches the padded A@V PSUM layout)."""
    pad = np.zeros((NQUAD, 4, 32, DM), np.float32)
    pad[:, :, :8, :] = W.T.reshape(NQUAD, 4, 8, DM)
    return np.ascontiguousarray(pad.reshape(NQUAD, 128, DM))


def _prep_core(inputs, bi, g):
    f = np.float32
    q = np.asarray(inputs["q"][bi], dtype=f)
    k = np.asarray(inputs["k"][bi], dtype=f)
    v = np.asarray(inputs["v"][bi], dtype=f)
    rows = slice(256 * g, 256 * (g + 1))
    zeros_dl = np.zeros((DM, L), f)
    return {
        "qT": np.ascontiguousarray(q.T),
        "kT": np.ascontiguousarray(k.T),
        "vT": np.ascontiguousarray(v.T),
        "simx": np.ascontiguousarray(np.asarray(inputs["similarity"][bi, 32 * g:32 * (g + 1)], dtype=f)),
        "wq": _pad_qk((np.asarray(inputs["Wq"], dtype=f)[rows] / TEMP).T),
        "wk": _pad_qk(np.asarray(inputs["Wk"], dtype=f)[rows].T),
        "wv": _pad_v(np.asarray(inputs["Wv"], dtype=f)[rows].T),
        "wfc": _pad_wfc(np.asarray(inputs["Wfc"], dtype=f)[:, rows]),
        "lng": np.ascontiguousarray(np.asarray(inputs["ln_g"], dtype=f)),
        "lnb": np.ascontiguousarray(np.asarray(inputs["ln_b"], dtype=f)),
        "bfc": np.ascontiguousarray(np.asarray(inputs["bfc"], dtype=f)) if g == 0 else np.zeros(DM, f),
        "resid": np.ascontiguousarray(q.T) if g == 0 else zeros_dl,
    }


def kernel(**inputs):
    nc = build_nc()
    in_maps = [_prep_core(inputs, c // HG, c % HG) for c in range(NCORES)]
    kw = {}
    if TRACE:
        kw = dict(trace=True, trace_cores=TRACE_CORES)
    res = run_bass_kernel_spmd(nc, in_maps, core_ids=list(range(NCORES)), **kw)
    LAST_RESULT["exec_time_ns"] = getattr(res, "exec_time_ns", None)
    LAST_RESULT["results"] = res
    outs = res.results

    attn = np.empty((B, DK, L, L), np.float32)
    out = np.empty((B, L, DM), np.float32)
    for c in range(NCORES):
        bi, g = c // HG, c % HG
        attn[bi, 32 * g:32 * (g + 1)] = outs[c]["attn_o"]
    for bi in range(B):
        acc = outs[HG * bi]["fc_o"].astype(np.float32).copy()
        for g in range(1, HG):
            acc += outs[HG * bi + g]["fc_o"]
        out[bi] = acc.T
    return out, attn
